# revision 49
# baseline (speedup 1.0000x reference)
"""Trainium2 Bass kernel for nn_CrossAttention (B=4, N=4096, T=256, DIM=1024,
16 heads x 64 dim, cosine-sim attention with null-kv token, LN in/ctx/out).

Sharding: data-parallel over query rows. Core c handles batch b=c//2, query
rows (c%2)*2048 : (c%2)*2048+2048. The kv projections (tiny: T=256) are
computed redundantly per core; no collectives are needed. Each core returns
its [2048, 1024] output slice (fp16); the host reassembles [4,4096,1024] f32.

Attention is computed with the AV matmul in [q, d] output orientation:
  scores[k, q] (keys on partitions) -> exp -> et (fp16, SBUF)
  po[q, h*64+d] = sum_k et[k, q] * v[k, d]   (lhsT = et block, K=128 keys,
  M=128 q, N=64) -- full PE utilization. The softmax denominator Z[q, h]
  rides N=1 ones-column matmuls, so 1/Z is a per-partition scalar multiply
  on DVE (no broadcast matmuls, no [64,512] reciprocals).
The null token contributes via per-head N=1 score matmuls, one
[16,16]-identity matmul folding exp(null) into Z, and a rank-16
block-diag(vnull) matmul folding exp(null)*vnull into po.

Emission is software-pipelined at macro (256-row) granularity with lag 3:
each driver iteration emits E/F(t-3) first (inputs all ready), then C/D(t-2),
then B(t-1), then A(t) -- every engine queue is in-order, so emission order
must be data-ready order per engine or the queue head blocks ready work.

Host-side input folds (pure input preprocessing, exact in f32):
  Wq' = diag(ln_in_g) Wq, qbias = ln_in_b @ Wq  (x-LN emits only (x-m)*rstd)
  Wkv' = diag(ln_ctx_g) Wkv, kvbias = ln_ctx_b @ Wkv
  knull = l2n(null_kv[0]) * (q_scale*k_scale), vnd = blockdiag16(null_kv[1])
  ln_out gain/bias applied on host (identity when g=1,b=0, the common case).
q_scale*k_scale is folded into the k side only; softmax needs no
max-subtraction (scores bounded by SCALE); exp uses fused scale=8,
bias=ln(1/256) which cancels in the division but keeps fp16 exps in range.
"""

import numpy as np
from contextlib import ExitStack

import concourse.bass as bass
import concourse.tile as tile
from concourse import bacc, mybir
from concourse.bass_utils import run_bass_kernel_spmd
from concourse.masks import make_identity

F32 = mybir.dt.float32
F16 = mybir.dt.float16
AF = mybir.ActivationFunctionType
AX = mybir.AxisListType

DIM = 1024
HEADS = 16
HD = 64
T = 256
SCALE = 8.0
EXPB = -5.545177444479562  # ln(1/256)
LN_EPS = 1e-5
NORM_EPS = 1e-12
N_CORES = 8
ROWS = 2048
MACRO = 256
NSUB = MACRO // 128      # 2
NMACRO = ROWS // MACRO   # 8
LAG = 3


def _emit_ln_stats(nc, pool_small, in_aps, eps_tile):
    """in_aps: list of 2 [128, 512] APs covering a 1024 row. Returns
    (rstd [128,1], negmr [128,1]) fp32 tiles for (x - m) * rstd.
    rsqrt computed as exp(-0.5*ln(var+eps)) -- ln/exp/identity/square share
    one activation table, so no ACT_TABLE_LOAD thrash (Sqrt does not)."""
    stats = pool_small.tile([128, 2, 6], F32, tag="lnstats", name="lnstats")
    for i, ap in enumerate(in_aps):
        nc.vector.bn_stats(out=stats[:, i, :], in_=ap)
    mv = pool_small.tile([128, 2], F32, tag="lnmv", name="lnmv")
    nc.vector.bn_aggr(out=mv[:], in_=stats[:])
    std = pool_small.tile([128, 1], F32, tag="lnstd", name="lnstd")
    nc.scalar.activation(out=std[:], in_=mv[:, 1:2], func=AF.Sqrt,
                         bias=eps_tile[:], scale=1.0)
    rstd = pool_small.tile([128, 1], F32, tag="lnrstd", name="lnrstd")
    nc.vector.reciprocal(rstd[:], std[:])
    negmr = pool_small.tile([128, 1], F32, tag="lnnegmr", name="lnnegmr")
    nc.vector.scalar_tensor_tensor(out=negmr[:], in0=mv[:, 0:1], scalar=-1.0,
                                   in1=rstd[:], op0=mybir.AluOpType.mult,
                                   op1=mybir.AluOpType.mult)
    return rstd, negmr


def _emit_l2norm_sq(nc, pool_small, sq_pool, in_half_aps):
    """Squares + per-head reduce (DVE only). Returns ssq [128,16] f16."""
    sq = sq_pool.tile([128, 1024], F16, tag="sq", name="sq")
    nc.scalar.activation(out=sq[:, 0:512], in_=in_half_aps[0], func=AF.Square,
                         bias=0.0, scale=1.0)
    nc.scalar.activation(out=sq[:, 512:1024], in_=in_half_aps[1], func=AF.Square,
                         bias=0.0, scale=1.0)
    ssq = pool_small.tile([128, 16], F16, tag="ssq", name="ssq")
    with nc.allow_low_precision(reason="l2 norm of 64 f16 squares; tol 2e-2"):
        nc.vector.reduce_sum(out=ssq[:],
                             in_=sq[:].rearrange("p (h d) -> p h d", d=HD),
                             axis=AX.X)
    return ssq


def _emit_l2norm_fin(nc, pool_small, out_ap3, in_half_aps, ssq, scale_tile):
    """sqrt/max/recip + normalize muls. out = in/max(||in_head||,eps)
    (* scale_tile [128,64] if given)."""
    norm = pool_small.tile([128, 16], F16, tag="l2norm", name="l2norm")
    nc.scalar.activation(out=norm[:], in_=ssq[:], func=AF.Sqrt,
                         bias=0.0, scale=1.0)
    nc.vector.tensor_scalar_max(norm[:], norm[:], NORM_EPS)
    rn = pool_small.tile([128, 16], F32, tag="l2rn", name="l2rn")
    nc.vector.reciprocal(rn[:], norm[:])
    for i in range(2):
        h0 = i * 8
        out_h = out_ap3[:, h0:h0 + 8, :]
        in3 = in_half_aps[i].rearrange("p (h d) -> p h d", d=HD)
        nc.vector.tensor_mul(
            out_h, in3,
            rn[:, h0:h0 + 8].unsqueeze(-1).broadcast_to([128, 8, HD]))
        if scale_tile is not None:
            nc.vector.tensor_mul(
                out_h, out_h,
                scale_tile[:].unsqueeze(1).broadcast_to([128, 8, HD]))


def _load_bcast(nc, dst_tile, dram_ap, parts=128):
    ap = bass.AP(tensor=dram_ap.tensor, offset=dram_ap.offset,
                 ap=[[0, parts]] + dram_ap.ap)
    nc.sync.dma_start(out=dst_tile[:parts, :], in_=ap)


def build_nc(zero_bias=False):
    nc = bacc.Bacc("TRN2", debug=False)

    XS = nc.dram_tensor("xs", [ROWS, DIM], F32, kind="ExternalInput")
    CTX = nc.dram_tensor("ctx", [T, DIM], F32, kind="ExternalInput")
    WQ = nc.dram_tensor("Wq", [DIM, DIM], F16, kind="ExternalInput")
    QB = nc.dram_tensor("qb", [1, DIM], F16, kind="ExternalInput")
    WKV = nc.dram_tensor("Wkv", [DIM, 2 * DIM], F16, kind="ExternalInput")
    KVB = nc.dram_tensor("kvb", [1, 2 * DIM], F16, kind="ExternalInput")
    WO = nc.dram_tensor("Wo", [DIM, DIM], F16, kind="ExternalInput")
    QK = nc.dram_tensor("qk", [HD], F32, kind="ExternalInput")
    KNULL = nc.dram_tensor("knull", [128, 2], F16, kind="ExternalInput")
    VND = nc.dram_tensor("vnd", [HEADS, DIM], F16, kind="ExternalInput")
    OUT = nc.dram_tensor("out", [ROWS, DIM], F16, kind="ExternalOutput")

    with tile.TileContext(nc) as tc, ExitStack() as ctx:
        consts = ctx.enter_context(tc.tile_pool(name="consts", bufs=1))
        weights = ctx.enter_context(tc.tile_pool(name="weights", bufs=1))
        kvpool = ctx.enter_context(tc.tile_pool(name="kvpool", bufs=1))
        small = ctx.enter_context(tc.tile_pool(name="small", bufs=8))
        sqp = ctx.enter_context(tc.tile_pool(name="sqp", bufs=4))

        # PSUM: mi(2) + pk(4) + po(2) = 8 banks
        ps_mi = ctx.enter_context(tc.tile_pool(name="ps_mi", bufs=2, space="PSUM"))
        ps_pk = ctx.enter_context(tc.tile_pool(name="ps_pk", bufs=4, space="PSUM"))
        ps_po = ctx.enter_context(tc.tile_pool(name="ps_po", bufs=2, space="PSUM"))

        ident = consts.tile([128, 128], F16, name="ident")
        make_identity(nc, ident)
        eps_tile = consts.tile([128, 1], F32, name="eps_tile")
        nc.vector.memset(eps_tile[:], LN_EPS)
        onesf = consts.tile([128, 1], F32, name="onesf")
        nc.vector.memset(onesf[:], 1.0)
        expb = consts.tile([128, 1], F32, name="expb")
        nc.vector.memset(expb[:], EXPB)
        eps24 = consts.tile([128, 1], F32, name="eps24")
        nc.vector.memset(eps24[:], NORM_EPS * NORM_EPS)
        ones_row = consts.tile([1, 128], F16, name="ones_row")
        nc.vector.tensor_copy(ones_row[0:1, :], onesf[0:1, 0:1].broadcast_to([1, 128]))
        ones_col = consts.tile([128, 1], F16, name="ones_col")
        nc.vector.tensor_copy(ones_col[:], onesf[:])

        # DMA order matters: ctx + wkv first (phase K), wq (first projections),
        # x tiles ride in per-iteration, wo last (first needed at iter LAG).
        qk_sb = consts.tile([128, HD], F32, name="qk_sb")
        _load_bcast(nc, qk_sb, QK[:])
        knull_sb = consts.tile([128, 2], F16, name="knull_sb")
        nc.sync.dma_start(out=knull_sb[:], in_=KNULL[:, :])
        vnd_sb = consts.tile([HEADS, DIM], F16, name="vnd_sb")
        nc.sync.dma_start(out=vnd_sb[0:HEADS, :], in_=VND[:, :])
        if not zero_bias:
            qb_sb = consts.tile([1, DIM], F16, name="qb_sb")
            nc.sync.dma_start(out=qb_sb[0:1, :], in_=QB[:, :])
            kvb_sb = consts.tile([1, 2 * DIM], F16, name="kvb_sb")
            nc.sync.dma_start(out=kvb_sb[0:1, :], in_=KVB[:, :])

        xin0 = ctx.enter_context(tc.tile_pool(name="xin", bufs=4))
        kT = kvpool.tile([128, 8, T], F16, name="kT")
        v_sb = kvpool.tile([128, 2, HEADS, HD], F16, name="v_sb")
        wq_sb = weights.tile([128, 8, DIM], F16, name="wq_sb")
        wo_sb = weights.tile([128, 8, DIM], F16, name="wo_sb")

        # ---------------- phase K: context -> kT, v ----------------
        kstack = ExitStack()
        pkp = kstack.enter_context(tc.tile_pool(name="pkp", bufs=2))
        pk1 = kstack.enter_context(tc.tile_pool(name="pk1", bufs=1))

        cnT = pk1.tile([128, 8, T], F16, name="cnT")
        cns = []
        for i in range(2):
            ctx_t = pkp.tile([128, DIM], F32, tag="ctx", name="ctx_t")
            for hf in range(2):
                nc.sync.dma_start(out=ctx_t[:, hf * 512:(hf + 1) * 512],
                                  in_=CTX[i * 128:(i + 1) * 128,
                                          hf * 512:(hf + 1) * 512])
            rstd, negmr = _emit_ln_stats(
                nc, small, [ctx_t[:, 0:512], ctx_t[:, 512:1024]], eps_tile)
            cn = pkp.tile([128, DIM], F16, tag="cn", name="cn")
            nc.scalar.activation(out=cn[:], in_=ctx_t[:], func=AF.Identity,
                                 bias=negmr[:], scale=rstd[:])
            cns.append(cn)

        st = {}

        def a_dma(m):
            s = st[m] = {}
            s["x"] = []
            for sub in range(NSUB):
                r0 = m * MACRO + sub * 128
                x_t = xin0.tile([128, DIM], F32, tag="x", name="x_t")
                for hf in range(2):
                    nc.sync.dma_start(out=x_t[:, hf * 512:(hf + 1) * 512],
                                      in_=XS[r0:r0 + 128,
                                             hf * 512:(hf + 1) * 512])
                s["x"].append(x_t)

        a_dma(0)
        wkv_sb = pk1.tile([128, 8, 2 * DIM], F16, name="wkv_sb")
        for kc in range(8):
            nc.sync.dma_start(out=wkv_sb[:, kc, :],
                              in_=WKV[kc * 128:(kc + 1) * 128, :])

        def cnt_t(i):
            ptr = ps_mi.tile([128, 8, 128], F16, tag="mi", name="ps_cnT")
            for t in range(8):
                nc.tensor.transpose(ptr[:, t, :],
                                    cns[i][:, t * 128:(t + 1) * 128], ident[:])
            nc.vector.tensor_copy(cnT[:, :, i * 128:(i + 1) * 128], ptr[:])

        def kv_mm(which, i, pool, tag):
            ph = [pool.tile([128, 512], F32, tag=tag, name=f"ph{h}")
                  for h in range(2)]
            if not zero_bias:
                for half in range(2):
                    col0 = which * DIM + half * 512
                    nc.tensor.matmul(ph[half][:], lhsT=ones_row[0:1, :],
                                     rhs=kvb_sb[0:1, col0:col0 + 512],
                                     start=True, stop=False)
            for kc in range(8):
                for half in range(2):
                    col0 = which * DIM + half * 512
                    nc.tensor.matmul(
                        ph[half][:],
                        lhsT=cnT[:, kc, i * 128:(i + 1) * 128],
                        rhs=wkv_sb[:, kc, col0:col0 + 512],
                        start=(zero_bias and kc == 0), stop=(kc == 7))
            return ph

        def k_fin(i, ph):
            kfin = pkp.tile([128, DIM], F16, tag="kfin", name="kfin")
            kssq = _emit_l2norm_sq(nc, small, sqp, [ph[0][:], ph[1][:]])
            _emit_l2norm_fin(nc, small,
                             kfin[:].rearrange("p (h d) -> p h d", d=HD),
                             [ph[0][:], ph[1][:]], kssq, qk_sb)
            ptr = ps_mi.tile([128, 8, 128], F16, tag="mi", name="ps_kT")
            for t in range(8):
                nc.tensor.transpose(ptr[:, t, :],
                                    kfin[:, t * 128:(t + 1) * 128], ident[:])
            nc.vector.tensor_copy(kT[:, :, i * 128:(i + 1) * 128], ptr[:])

        def v_fin(i, ph):
            for half in range(2):
                nc.vector.tensor_copy(
                    v_sb[:, i, half * 8:(half + 1) * 8, :],
                    ph[half][:].rearrange("p (h d) -> p h d", d=HD))

        cnt_t(0)
        phk0 = kv_mm(0, 0, ps_pk, "pk")
        cnt_t(1)
        phk1 = kv_mm(0, 1, ps_pk, "pk")
        phv0 = kv_mm(1, 0, ps_po, "po")
        k_fin(0, phk0)
        phv1 = kv_mm(1, 1, ps_po, "po")
        k_fin(1, phk1)
        v_fin(0, phv0)
        v_fin(1, phv1)

        for kc in range(8):
            nc.sync.dma_start(out=wq_sb[:, kc, :], in_=WQ[kc * 128:(kc + 1) * 128, :])

        kstack.close()

        # ---------------- main loop pools ----------------
        xin = xin0
        xnp = ctx.enter_context(tc.tile_pool(name="xnp", bufs=4))
        xnTs = ctx.enter_context(tc.tile_pool(name="xnTs", bufs=4))
        qfp = ctx.enter_context(tc.tile_pool(name="qfp", bufs=6))
        qTp = ctx.enter_context(tc.tile_pool(name="qTp", bufs=3))
        etp = ctx.enter_context(tc.tile_pool(name="etp", bufs=2))
        enp = ctx.enter_context(tc.tile_pool(name="enp", bufs=4))
        enTs = ctx.enter_context(tc.tile_pool(name="enTs", bufs=4))
        rzp = ctx.enter_context(tc.tile_pool(name="rzp", bufs=8))
        aop = ctx.enter_context(tc.tile_pool(name="aop", bufs=4))
        aTp = ctx.enter_context(tc.tile_pool(name="aTp", bufs=4))
        obp = ctx.enter_context(tc.tile_pool(name="obp", bufs=4))

        i16 = ident[0:HEADS, 0:HEADS]

        def a_stats(m):
            st[m]["stats"] = []
            for sub in range(NSUB):
                x_t = st[m]["x"][sub]
                st[m]["stats"].append(_emit_ln_stats(
                    nc, small, [x_t[:, 0:512], x_t[:, 512:1024]], eps_tile))

        def a_act(m):
            st[m]["xn"] = []
            for sub in range(NSUB):
                rstd, negmr = st[m]["stats"][sub]
                xn = xnp.tile([128, DIM], F16, tag="xn", name="xn")
                nc.scalar.activation(out=xn[:], in_=st[m]["x"][sub], func=AF.Identity,
                                     bias=negmr[:], scale=rstd[:])
                st[m]["xn"].append(xn)

        def b1(m, sub):
            xn = st[m]["xn"][sub]
            ptr = ps_mi.tile([128, 8, 128], F16, tag="mi", name="ps_xnT")
            for t in range(8):
                nc.tensor.transpose(ptr[:, t, :],
                                    xn[:, t * 128:(t + 1) * 128], ident[:])
            xnT = xnTs.tile([128, 8, 128], F16, tag="xnT", name="xnT")
            nc.vector.tensor_copy(xnT[:], ptr[:])
            st[m][f"xnT{sub}"] = xnT

        def b2(m, sub):
            xnT = st[m][f"xnT{sub}"]
            pq = [ps_pk.tile([128, 512], F32, tag="pk", name=f"pq{w}")
                  for w in range(2)]
            for half in range(2):
                if not zero_bias:
                    nc.tensor.matmul(pq[half][:], lhsT=ones_row[0:1, :],
                                     rhs=qb_sb[0:1, half * 512:(half + 1) * 512],
                                     start=True, stop=False)
                for kc in range(8):
                    nc.tensor.matmul(
                        pq[half][:], lhsT=xnT[:, kc, :],
                        rhs=wq_sb[:, kc, half * 512:(half + 1) * 512],
                        start=(zero_bias and kc == 0), stop=(kc == 7))
            st[m][f"pq{sub}"] = pq
            st[m][f"ssq{sub}"] = _emit_l2norm_sq(nc, small, sqp,
                                                 [pq[0][:], pq[1][:]])

        def b2f(m, sub):
            pq = st[m][f"pq{sub}"]
            qf = qfp.tile([128, DIM], F16, tag="qf", name="qf")
            _emit_l2norm_fin(nc, small,
                             qf[:].rearrange("p (h d) -> p h d", d=HD),
                             [pq[0][:], pq[1][:]], st[m][f"ssq{sub}"], None)
            st[m][f"qf{sub}"] = qf

        def b3(m, sub):
            if sub == 0:
                st[m]["qT"] = qTp.tile([128, 8, MACRO], F16, tag="qT", name="qT")
            qf = st[m][f"qf{sub}"]
            ptr = ps_mi.tile([128, 8, 128], F16, tag="mi", name="ps_qT")
            for t in range(8):
                nc.tensor.transpose(ptr[:, t, :],
                                    qf[:, t * 128:(t + 1) * 128], ident[:])
            nc.vector.tensor_copy(st[m]["qT"][:, :, sub * 128:(sub + 1) * 128], ptr[:])

        def c_mm(m, sub):
            qT = st[m]["qT"]
            ps_ns = ps_mi.tile([128, HEADS], F32, tag="mi", name="ps_ns")
            for ch in range(8):
                nc.tensor.matmul(
                    ps_ns[:, 2 * ch:2 * ch + 2],
                    lhsT=qT[:, ch, sub * 128:(sub + 1) * 128],
                    rhs=knull_sb[:, :], start=True,
                    stop=True, skip_group_check=True)
            st[m][f"ns{sub}"] = ps_ns

        def c_exp(m, sub):
            en = enp.tile([128, HEADS], F16, tag="en", name="en")
            nc.scalar.activation(out=en[:], in_=st[m][f"ns{sub}"], func=AF.Exp,
                                 bias=expb[:], scale=SCALE)
            st[m][f"en{sub}"] = en

        def c_t(m, sub):
            ps_ent = ps_mi.tile([HEADS, 128], F16, tag="mi", name="ps_ent")
            nc.tensor.transpose(ps_ent[:], st[m][f"en{sub}"], ident[:])
            enT = enTs.tile([HEADS, 128], F16, tag="enT", name="enT")
            nc.vector.tensor_copy(enT[:], ps_ent[:])
            st[m][f"enT{sub}"] = enT

        def d_stage(m, h0, h1):
            if h0 == 0:
                st[m]["et"] = etp.tile([128, 2, HEADS, MACRO], F16, tag="et",
                                       name="et")
            et, qT = st[m]["et"], st[m]["qT"]
            for h in range(h0, h1):
                jb, ch = (h % 2) * HD, h // 2
                sc = ps_pk.tile([128, 2, MACRO], F32, tag="pk", name="sc")
                for kb in range(2):
                    nc.tensor.matmul(
                        sc[:, kb, :],
                        lhsT=kT[jb:jb + HD, ch, kb * 128:(kb + 1) * 128],
                        rhs=qT[jb:jb + HD, ch, :], start=True,
                        stop=True, skip_group_check=True)
                nc.scalar.activation(out=et[:, :, h, :], in_=sc[:], func=AF.Exp,
                                     bias=expb[:], scale=SCALE)

        def e_av(m, sub):
            et, enT = st[m]["et"], st[m][f"enT{sub}"]
            zn = ps_mi.tile([128, HEADS], F32, tag="mi", name="zn")
            for h in range(HEADS):
                for kb in range(2):
                    nc.tensor.matmul(
                        zn[:, h:h + 1],
                        lhsT=et[:, kb, h, sub * 128:(sub + 1) * 128],
                        rhs=ones_col[:, :], start=(kb == 0),
                        stop=(kb == 1), skip_group_check=True)
            z = rzp.tile([128, HEADS], F32, tag="z", name="z")
            nc.vector.tensor_add(z[:], zn[:], st[m][f"en{sub}"])
            rz = rzp.tile([128, HEADS], F32, tag="rz", name="rz")
            nc.vector.reciprocal(rz[:], z[:])
            ao = aop.tile([128, HEADS, HD], F16, tag="ao", name="ao")
            for half in range(2):
                po = ps_po.tile([128, 8, HD], F32, tag="po", name=f"po{half}")
                for hh in range(8):
                    h = half * 8 + hh
                    for kb in range(2):
                        nc.tensor.matmul(
                            po[:, hh, :],
                            lhsT=et[:, kb, h, sub * 128:(sub + 1) * 128],
                            rhs=v_sb[:, kb, h, :], start=(kb == 0),
                            stop=False, skip_group_check=True)
                    nc.tensor.matmul(
                        po[:, hh, :], lhsT=enT[:, :],
                        rhs=vnd_sb[0:HEADS, h * HD:(h + 1) * HD],
                        start=False, stop=True, skip_group_check=True)
                nc.vector.tensor_mul(
                    ao[:, half * 8:(half + 1) * 8, :], po[:],
                    rz[:, half * 8:(half + 1) * 8].unsqueeze(-1)
                    .broadcast_to([128, 8, HD]))
            st[m][f"ao{sub}"] = ao

        def e_t(m, sub):
            ao_flat = st[m][f"ao{sub}"][:].rearrange("p h d -> p (h d)")
            ptr = ps_mi.tile([128, 8, 128], F16, tag="mi", name="ps_aT")
            for t in range(8):
                nc.tensor.transpose(ptr[:, t, :],
                                    ao_flat[:, t * 128:(t + 1) * 128], ident[:])
            aT = aTp.tile([128, 8, 128], F16, tag="aT", name="aT")
            nc.vector.tensor_copy(aT[:], ptr[:])
            st[m][f"aT{sub}"] = aT

        def f_stage(m, sub):
            aT = st[m][f"aT{sub}"]
            r0 = m * MACRO + sub * 128
            pf = [ps_pk.tile([128, 512], F32, tag="pk", name=f"pf{w}")
                  for w in range(2)]
            for half in range(2):
                for kc in range(8):
                    nc.tensor.matmul(
                        pf[half][:], lhsT=aT[:, kc, :],
                        rhs=wo_sb[:, kc, half * 512:(half + 1) * 512],
                        start=(kc == 0), stop=(kc == 7))
            rstd, negmr = _emit_ln_stats(nc, small, [pf[0][:], pf[1][:]], eps_tile)
            ob = obp.tile([128, DIM], F16, tag="ob", name="ob")
            for half in range(2):
                nc.scalar.activation(out=ob[:, half * 512:(half + 1) * 512],
                                     in_=pf[half][:], func=AF.Identity,
                                     bias=negmr[:], scale=rstd[:])
                nc.sync.dma_start(
                    out=OUT[r0:r0 + 128, half * 512:(half + 1) * 512],
                    in_=ob[:, half * 512:(half + 1) * 512])

        for t in range(NMACRO + LAG):
            if 0 < t < NMACRO:
                a_dma(t)
            if t == 1:
                for kc in range(8):
                    nc.sync.dma_start(out=wo_sb[:, kc, :],
                                      in_=WO[kc * 128:(kc + 1) * 128, :])
            do_b = 1 <= t <= NMACRO
            do_b3 = 2 <= t <= NMACRO + 1
            do_cd = 2 <= t <= NMACRO + 1
            do_ef = t >= LAG
            if do_b3:
                b3(t - 2, 0)
            if do_ef:
                c_t(t - 3, 0)
                c_t(t - 3, 1)
            if do_b3:
                b3(t - 2, 1)
            if do_b:
                b1(t - 1, 0)
                b1(t - 1, 1)
            if do_cd:
                c_mm(t - 2, 0)
                c_mm(t - 2, 1)
                c_exp(t - 2, 0)
                c_exp(t - 2, 1)
            if do_b:
                b2(t - 1, 0)
                b2(t - 1, 1)
                b2f(t - 1, 0)
                b2f(t - 1, 1)
            if do_cd:
                d_stage(t - 2, 0, 16)
            if do_ef:
                e_av(t - 3, 0)
                e_t(t - 3, 0)
                e_av(t - 3, 1)
                e_t(t - 3, 1)
            if t < NMACRO:
                a_stats(t)
                a_act(t)
            if do_ef:
                f_stage(t - 3, 0)
                f_stage(t - 3, 1)
            if t - 3 - 1 in st:
                del st[t - 3 - 1]

    nc.compile()
    return nc


_NC_BUILDS = {}
_NC_CACHE = None  # last-used module (kept for external tooling/harness)


def kernel(**inputs):
    global _NC_CACHE

    x = np.asarray(inputs["x"], np.float32)
    context = np.asarray(inputs["context"], np.float32)
    Wq = np.asarray(inputs["Wq"], np.float32)
    Wkv = np.asarray(inputs["Wkv"], np.float32)
    Wo = np.asarray(inputs["Wo"], np.float32)
    null_kv = np.asarray(inputs["null_kv"], np.float32)
    q_scale = np.asarray(inputs["q_scale"], np.float32)
    k_scale = np.asarray(inputs["k_scale"], np.float32)
    lig = np.asarray(inputs["ln_in_g"], np.float32)
    lib = np.asarray(inputs["ln_in_b"], np.float32)
    lcg = np.asarray(inputs["ln_ctx_g"], np.float32)
    lcb = np.asarray(inputs["ln_ctx_b"], np.float32)
    log_ = np.asarray(inputs["ln_out_g"], np.float32)
    lob = np.asarray(inputs["ln_out_b"], np.float32)

    qb_np = (lib @ Wq).astype(np.float32)
    kvb_np = (lcb @ Wkv).astype(np.float32)
    zb = not (np.any(qb_np) or np.any(kvb_np))
    if zb not in _NC_BUILDS:
        _NC_BUILDS[zb] = build_nc(zero_bias=zb)
    nc = _NC_CACHE = _NC_BUILDS[zb]

    qk = q_scale * k_scale
    kn = null_kv[0]
    kn = kn / max(np.sqrt((kn * kn).sum()), NORM_EPS) * qk
    knull = np.zeros((128, 2), np.float16)
    knull[0:HD, 0] = kn.astype(np.float16)
    knull[HD:128, 1] = kn.astype(np.float16)
    vnd = np.zeros((HEADS, DIM), np.float16)
    for h in range(HEADS):
        vnd[h, h * HD:(h + 1) * HD] = null_kv[1].astype(np.float16)

    shared = {
        "Wq": (lig[:, None] * Wq).astype(np.float16),
        "qb": qb_np[None, :].astype(np.float16),
        "Wkv": (lcg[:, None] * Wkv).astype(np.float16),
        "kvb": kvb_np[None, :].astype(np.float16),
        "Wo": Wo.astype(np.float16),
        "qk": qk,
        "knull": knull,
        "vnd": vnd,
    }
    B, N, _ = x.shape
    in_maps = []
    for c in range(N_CORES):
        b, n0 = c // 2, (c % 2) * ROWS
        in_maps.append({"xs": np.ascontiguousarray(x[b, n0:n0 + ROWS]),
                        "ctx": np.ascontiguousarray(context[b]), **shared})

    res = run_bass_kernel_spmd(nc, in_maps, list(range(N_CORES)))

    out = np.empty((B, N, DIM), np.float32)
    for c in range(N_CORES):
        b, n0 = c // 2, (c % 2) * ROWS
        out[b, n0:n0 + ROWS] = res.results[c]["out"].astype(np.float32)
    # ln_out gain/bias applied host-side (identity for g=1, b=0).
    if not (np.all(log_ == 1.0) and np.all(lob == 0.0)):
        out = out * log_ + lob
    return out


# revision 50
# speedup vs baseline: 1.0040x; 1.0040x over previous
"""Trainium2 Bass kernel for nn_CrossAttention (B=4, N=4096, T=256, DIM=1024,
16 heads x 64 dim, cosine-sim attention with null-kv token, LN in/ctx/out).

Sharding: data-parallel over query rows. Core c handles batch b=c//2, query
rows (c%2)*2048 : (c%2)*2048+2048. The kv projections (tiny: T=256) are
computed redundantly per core; no collectives are needed. Each core returns
its [2048, 1024] output slice (fp16); the host reassembles [4,4096,1024] f32.

Attention is computed with the AV matmul in [q, d] output orientation:
  scores[k, q] (keys on partitions) -> exp -> et (fp16, SBUF)
  po[q, h*64+d] = sum_k et[k, q] * v[k, d]   (lhsT = et block, K=128 keys,
  M=128 q, N=64) -- full PE utilization. The softmax denominator Z[q, h]
  rides N=1 ones-column matmuls, so 1/Z is a per-partition scalar multiply
  on DVE (no broadcast matmuls, no [64,512] reciprocals).
The null token contributes via per-head N=1 score matmuls, one
[16,16]-identity matmul folding exp(null) into Z, and a rank-16
block-diag(vnull) matmul folding exp(null)*vnull into po.

Emission is software-pipelined at macro (256-row) granularity with lag 3:
each driver iteration emits E/F(t-3) first (inputs all ready), then C/D(t-2),
then B(t-1), then A(t) -- every engine queue is in-order, so emission order
must be data-ready order per engine or the queue head blocks ready work.

Host-side input folds (pure input preprocessing, exact in f32):
  Wq' = diag(ln_in_g) Wq, qbias = ln_in_b @ Wq  (x-LN emits only (x-m)*rstd)
  Wkv' = diag(ln_ctx_g) Wkv, kvbias = ln_ctx_b @ Wkv
  knull = l2n(null_kv[0]) * (q_scale*k_scale), vnd = blockdiag16(null_kv[1])
  ln_out gain/bias applied on host (identity when g=1,b=0, the common case).
q_scale*k_scale is folded into the k side only; softmax needs no
max-subtraction (scores bounded by SCALE); exp uses fused scale=8,
bias=ln(1/256) which cancels in the division but keeps fp16 exps in range.
"""

import numpy as np
from contextlib import ExitStack

import concourse.bass as bass
import concourse.tile as tile
from concourse import bacc, mybir
from concourse.bass_utils import run_bass_kernel_spmd
from concourse.masks import make_identity

F32 = mybir.dt.float32
F16 = mybir.dt.float16
AF = mybir.ActivationFunctionType
AX = mybir.AxisListType

DIM = 1024
HEADS = 16
HD = 64
T = 256
SCALE = 8.0
EXPB = -5.545177444479562  # ln(1/256)
LN_EPS = 1e-5
NORM_EPS = 1e-12
N_CORES = 8
ROWS = 2048
MACRO = 256
NSUB = MACRO // 128      # 2
NMACRO = ROWS // MACRO   # 8
LAG = 3


def _emit_ln_stats(nc, pool_small, in_aps, eps_tile):
    """in_aps: list of 2 [128, 512] APs covering a 1024 row. Returns
    (rstd [128,1], negmr [128,1]) fp32 tiles for (x - m) * rstd.
    rsqrt computed as exp(-0.5*ln(var+eps)) -- ln/exp/identity/square share
    one activation table, so no ACT_TABLE_LOAD thrash (Sqrt does not)."""
    stats = pool_small.tile([128, 2, 6], F32, tag="lnstats", name="lnstats")
    for i, ap in enumerate(in_aps):
        nc.vector.bn_stats(out=stats[:, i, :], in_=ap)
    mv = pool_small.tile([128, 2], F32, tag="lnmv", name="lnmv")
    nc.vector.bn_aggr(out=mv[:], in_=stats[:])
    std = pool_small.tile([128, 1], F32, tag="lnstd", name="lnstd")
    nc.scalar.activation(out=std[:], in_=mv[:, 1:2], func=AF.Sqrt,
                         bias=eps_tile[:], scale=1.0)
    rstd = pool_small.tile([128, 1], F32, tag="lnrstd", name="lnrstd")
    nc.vector.reciprocal(rstd[:], std[:])
    negmr = pool_small.tile([128, 1], F32, tag="lnnegmr", name="lnnegmr")
    nc.vector.scalar_tensor_tensor(out=negmr[:], in0=mv[:, 0:1], scalar=-1.0,
                                   in1=rstd[:], op0=mybir.AluOpType.mult,
                                   op1=mybir.AluOpType.mult)
    return rstd, negmr


def _emit_l2norm_sq(nc, pool_small, sq_pool, in_half_aps):
    """Squares + per-head reduce (DVE only). Returns ssq [128,16] f16."""
    sq = sq_pool.tile([128, 1024], F16, tag="sq", name="sq")
    nc.scalar.activation(out=sq[:, 0:512], in_=in_half_aps[0], func=AF.Square,
                         bias=0.0, scale=1.0)
    nc.scalar.activation(out=sq[:, 512:1024], in_=in_half_aps[1], func=AF.Square,
                         bias=0.0, scale=1.0)
    ssq = pool_small.tile([128, 16], F16, tag="ssq", name="ssq")
    with nc.allow_low_precision(reason="l2 norm of 64 f16 squares; tol 2e-2"):
        nc.vector.reduce_sum(out=ssq[:],
                             in_=sq[:].rearrange("p (h d) -> p h d", d=HD),
                             axis=AX.X)
    return ssq


def _emit_l2norm_fin(nc, pool_small, out_ap3, in_half_aps, ssq, scale_tile):
    """sqrt/max/recip + normalize muls. out = in/max(||in_head||,eps)
    (* scale_tile [128,64] if given)."""
    norm = pool_small.tile([128, 16], F16, tag="l2norm", name="l2norm")
    nc.scalar.activation(out=norm[:], in_=ssq[:], func=AF.Sqrt,
                         bias=0.0, scale=1.0)
    nc.vector.tensor_scalar_max(norm[:], norm[:], NORM_EPS)
    rn = pool_small.tile([128, 16], F32, tag="l2rn", name="l2rn")
    nc.vector.reciprocal(rn[:], norm[:])
    for i in range(2):
        h0 = i * 8
        out_h = out_ap3[:, h0:h0 + 8, :]
        in3 = in_half_aps[i].rearrange("p (h d) -> p h d", d=HD)
        nc.vector.tensor_mul(
            out_h, in3,
            rn[:, h0:h0 + 8].unsqueeze(-1).broadcast_to([128, 8, HD]))
        if scale_tile is not None:
            nc.vector.tensor_mul(
                out_h, out_h,
                scale_tile[:].unsqueeze(1).broadcast_to([128, 8, HD]))


def _load_bcast(nc, dst_tile, dram_ap, parts=128):
    ap = bass.AP(tensor=dram_ap.tensor, offset=dram_ap.offset,
                 ap=[[0, parts]] + dram_ap.ap)
    nc.sync.dma_start(out=dst_tile[:parts, :], in_=ap)


def build_nc(zero_bias=False):
    nc = bacc.Bacc("TRN2", debug=False)

    XS = nc.dram_tensor("xs", [ROWS, DIM], F32, kind="ExternalInput")
    CTX = nc.dram_tensor("ctx", [T, DIM], F32, kind="ExternalInput")
    WQ = nc.dram_tensor("Wq", [DIM, DIM], F16, kind="ExternalInput")
    QB = nc.dram_tensor("qb", [1, DIM], F16, kind="ExternalInput")
    WKV = nc.dram_tensor("Wkv", [DIM, 2 * DIM], F16, kind="ExternalInput")
    KVB = nc.dram_tensor("kvb", [1, 2 * DIM], F16, kind="ExternalInput")
    WO = nc.dram_tensor("Wo", [DIM, DIM], F16, kind="ExternalInput")
    QK = nc.dram_tensor("qk", [HD], F32, kind="ExternalInput")
    KNULL = nc.dram_tensor("knull", [128, 2], F16, kind="ExternalInput")
    VND = nc.dram_tensor("vnd", [HEADS, DIM], F16, kind="ExternalInput")
    OUT = nc.dram_tensor("out", [ROWS, DIM], F16, kind="ExternalOutput")

    with tile.TileContext(nc) as tc, ExitStack() as ctx:
        consts = ctx.enter_context(tc.tile_pool(name="consts", bufs=1))
        weights = ctx.enter_context(tc.tile_pool(name="weights", bufs=1))
        kvpool = ctx.enter_context(tc.tile_pool(name="kvpool", bufs=1))
        small = ctx.enter_context(tc.tile_pool(name="small", bufs=8))
        sqp = ctx.enter_context(tc.tile_pool(name="sqp", bufs=4))

        # PSUM: mi(2) + pk(4) + po(2) = 8 banks
        ps_mi = ctx.enter_context(tc.tile_pool(name="ps_mi", bufs=2, space="PSUM"))
        ps_pk = ctx.enter_context(tc.tile_pool(name="ps_pk", bufs=4, space="PSUM"))
        ps_po = ctx.enter_context(tc.tile_pool(name="ps_po", bufs=2, space="PSUM"))

        ident = consts.tile([128, 128], F16, name="ident")
        make_identity(nc, ident)
        eps_tile = consts.tile([128, 1], F32, name="eps_tile")
        nc.vector.memset(eps_tile[:], LN_EPS)
        onesf = consts.tile([128, 1], F32, name="onesf")
        nc.vector.memset(onesf[:], 1.0)
        expb = consts.tile([128, 1], F32, name="expb")
        nc.vector.memset(expb[:], EXPB)
        eps24 = consts.tile([128, 1], F32, name="eps24")
        nc.vector.memset(eps24[:], NORM_EPS * NORM_EPS)
        ones_row = consts.tile([1, 128], F16, name="ones_row")
        nc.vector.tensor_copy(ones_row[0:1, :], onesf[0:1, 0:1].broadcast_to([1, 128]))
        ones_col = consts.tile([128, 1], F16, name="ones_col")
        nc.vector.tensor_copy(ones_col[:], onesf[:])

        # DMA order matters: ctx + wkv first (phase K), wq (first projections),
        # x tiles ride in per-iteration, wo last (first needed at iter LAG).
        qk_sb = consts.tile([128, HD], F32, name="qk_sb")
        _load_bcast(nc, qk_sb, QK[:])
        knull_sb = consts.tile([128, 2], F16, name="knull_sb")
        nc.sync.dma_start(out=knull_sb[:], in_=KNULL[:, :])
        vnd_sb = consts.tile([HEADS, DIM], F16, name="vnd_sb")
        nc.sync.dma_start(out=vnd_sb[0:HEADS, :], in_=VND[:, :])
        if not zero_bias:
            qb_sb = consts.tile([1, DIM], F16, name="qb_sb")
            nc.sync.dma_start(out=qb_sb[0:1, :], in_=QB[:, :])
            kvb_sb = consts.tile([1, 2 * DIM], F16, name="kvb_sb")
            nc.sync.dma_start(out=kvb_sb[0:1, :], in_=KVB[:, :])

        xin0 = ctx.enter_context(tc.tile_pool(name="xin", bufs=4))
        kT = kvpool.tile([128, 8, T], F16, name="kT")
        v_sb = kvpool.tile([128, 2, HEADS, HD], F16, name="v_sb")
        wq_sb = weights.tile([128, 8, DIM], F16, name="wq_sb")
        wo_sb = weights.tile([128, 8, DIM], F16, name="wo_sb")

        # ---------------- phase K: context -> kT, v ----------------
        kstack = ExitStack()
        pkp = kstack.enter_context(tc.tile_pool(name="pkp", bufs=2))
        pk1 = kstack.enter_context(tc.tile_pool(name="pk1", bufs=1))

        cnT = pk1.tile([128, 8, T], F16, name="cnT")
        cns = []
        for i in range(2):
            ctx_t = pkp.tile([128, DIM], F32, tag="ctx", name="ctx_t")
            for hf in range(2):
                nc.sync.dma_start(out=ctx_t[:, hf * 512:(hf + 1) * 512],
                                  in_=CTX[i * 128:(i + 1) * 128,
                                          hf * 512:(hf + 1) * 512])
            rstd, negmr = _emit_ln_stats(
                nc, small, [ctx_t[:, 0:512], ctx_t[:, 512:1024]], eps_tile)
            cn = pkp.tile([128, DIM], F16, tag="cn", name="cn")
            nc.scalar.activation(out=cn[:], in_=ctx_t[:], func=AF.Identity,
                                 bias=negmr[:], scale=rstd[:])
            cns.append(cn)

        st = {}

        def a_dma(m):
            s = st[m] = {}
            s["x"] = []
            for sub in range(NSUB):
                r0 = m * MACRO + sub * 128
                x_t = xin0.tile([128, DIM], F32, tag="x", name="x_t")
                for hf in range(2):
                    nc.sync.dma_start(out=x_t[:, hf * 512:(hf + 1) * 512],
                                      in_=XS[r0:r0 + 128,
                                             hf * 512:(hf + 1) * 512])
                s["x"].append(x_t)

        a_dma(0)
        wkv_sb = pk1.tile([128, 8, 2 * DIM], F16, name="wkv_sb")
        for kc in range(8):
            nc.sync.dma_start(out=wkv_sb[:, kc, :],
                              in_=WKV[kc * 128:(kc + 1) * 128, :])

        def cnt_t(i):
            ptr = ps_mi.tile([128, 8, 128], F16, tag="mi", name="ps_cnT")
            for t in range(8):
                nc.tensor.transpose(ptr[:, t, :],
                                    cns[i][:, t * 128:(t + 1) * 128], ident[:])
            nc.vector.tensor_copy(cnT[:, :, i * 128:(i + 1) * 128], ptr[:])

        def kv_mm(which, i, pool, tag):
            ph = [pool.tile([128, 512], F32, tag=tag, name=f"ph{h}")
                  for h in range(2)]
            if not zero_bias:
                for half in range(2):
                    col0 = which * DIM + half * 512
                    nc.tensor.matmul(ph[half][:], lhsT=ones_row[0:1, :],
                                     rhs=kvb_sb[0:1, col0:col0 + 512],
                                     start=True, stop=False)
            for kc in range(8):
                for half in range(2):
                    col0 = which * DIM + half * 512
                    nc.tensor.matmul(
                        ph[half][:],
                        lhsT=cnT[:, kc, i * 128:(i + 1) * 128],
                        rhs=wkv_sb[:, kc, col0:col0 + 512],
                        start=(zero_bias and kc == 0), stop=(kc == 7))
            return ph

        def k_fin(i, ph):
            kfin = pkp.tile([128, DIM], F16, tag="kfin", name="kfin")
            kssq = _emit_l2norm_sq(nc, small, sqp, [ph[0][:], ph[1][:]])
            _emit_l2norm_fin(nc, small,
                             kfin[:].rearrange("p (h d) -> p h d", d=HD),
                             [ph[0][:], ph[1][:]], kssq, qk_sb)
            ptr = ps_mi.tile([128, 8, 128], F16, tag="mi", name="ps_kT")
            for t in range(8):
                nc.tensor.transpose(ptr[:, t, :],
                                    kfin[:, t * 128:(t + 1) * 128], ident[:])
            nc.vector.tensor_copy(kT[:, :, i * 128:(i + 1) * 128], ptr[:])

        def v_fin(i, ph):
            for half in range(2):
                nc.vector.tensor_copy(
                    v_sb[:, i, half * 8:(half + 1) * 8, :],
                    ph[half][:].rearrange("p (h d) -> p h d", d=HD))

        cnt_t(0)
        phk0 = kv_mm(0, 0, ps_pk, "pk")
        cnt_t(1)
        phk1 = kv_mm(0, 1, ps_pk, "pk")
        phv0 = kv_mm(1, 0, ps_po, "po")
        k_fin(0, phk0)
        phv1 = kv_mm(1, 1, ps_po, "po")
        k_fin(1, phk1)
        v_fin(0, phv0)
        v_fin(1, phv1)

        for kc in range(8):
            nc.sync.dma_start(out=wq_sb[:, kc, :], in_=WQ[kc * 128:(kc + 1) * 128, :])

        kstack.close()

        # ---------------- main loop pools ----------------
        xin = xin0
        xnp = ctx.enter_context(tc.tile_pool(name="xnp", bufs=4))
        xnTs = ctx.enter_context(tc.tile_pool(name="xnTs", bufs=4))
        qfp = ctx.enter_context(tc.tile_pool(name="qfp", bufs=6))
        qTp = ctx.enter_context(tc.tile_pool(name="qTp", bufs=3))
        etp = ctx.enter_context(tc.tile_pool(name="etp", bufs=2))
        enp = ctx.enter_context(tc.tile_pool(name="enp", bufs=4))
        enTs = ctx.enter_context(tc.tile_pool(name="enTs", bufs=4))
        rzp = ctx.enter_context(tc.tile_pool(name="rzp", bufs=8))
        aop = ctx.enter_context(tc.tile_pool(name="aop", bufs=4))
        aTp = ctx.enter_context(tc.tile_pool(name="aTp", bufs=4))
        obp = ctx.enter_context(tc.tile_pool(name="obp", bufs=4))

        i16 = ident[0:HEADS, 0:HEADS]

        def a_stats(m):
            st[m]["stats"] = []
            for sub in range(NSUB):
                x_t = st[m]["x"][sub]
                st[m]["stats"].append(_emit_ln_stats(
                    nc, small, [x_t[:, 0:512], x_t[:, 512:1024]], eps_tile))

        def a_act(m):
            st[m]["xn"] = []
            for sub in range(NSUB):
                rstd, negmr = st[m]["stats"][sub]
                xn = xnp.tile([128, DIM], F16, tag="xn", name="xn")
                nc.scalar.activation(out=xn[:], in_=st[m]["x"][sub], func=AF.Identity,
                                     bias=negmr[:], scale=rstd[:])
                st[m]["xn"].append(xn)

        def b1(m, sub):
            xn = st[m]["xn"][sub]
            ptr = ps_mi.tile([128, 8, 128], F16, tag="mi", name="ps_xnT")
            for t in range(8):
                nc.tensor.transpose(ptr[:, t, :],
                                    xn[:, t * 128:(t + 1) * 128], ident[:])
            xnT = xnTs.tile([128, 8, 128], F16, tag="xnT", name="xnT")
            nc.vector.tensor_copy(xnT[:], ptr[:])
            st[m][f"xnT{sub}"] = xnT

        def b2(m, sub):
            xnT = st[m][f"xnT{sub}"]
            pq = [ps_pk.tile([128, 512], F32, tag="pk", name=f"pq{w}")
                  for w in range(2)]
            for half in range(2):
                if not zero_bias:
                    nc.tensor.matmul(pq[half][:], lhsT=ones_row[0:1, :],
                                     rhs=qb_sb[0:1, half * 512:(half + 1) * 512],
                                     start=True, stop=False)
                for kc in range(8):
                    nc.tensor.matmul(
                        pq[half][:], lhsT=xnT[:, kc, :],
                        rhs=wq_sb[:, kc, half * 512:(half + 1) * 512],
                        start=(zero_bias and kc == 0), stop=(kc == 7))
            st[m][f"pq{sub}"] = pq
            st[m][f"ssq{sub}"] = _emit_l2norm_sq(nc, small, sqp,
                                                 [pq[0][:], pq[1][:]])

        def b2f(m, sub):
            pq = st[m][f"pq{sub}"]
            qf = qfp.tile([128, DIM], F16, tag="qf", name="qf")
            _emit_l2norm_fin(nc, small,
                             qf[:].rearrange("p (h d) -> p h d", d=HD),
                             [pq[0][:], pq[1][:]], st[m][f"ssq{sub}"], None)
            st[m][f"qf{sub}"] = qf

        def b3(m, sub):
            if sub == 0:
                st[m]["qT"] = qTp.tile([128, 8, MACRO], F16, tag="qT", name="qT")
            qf = st[m][f"qf{sub}"]
            ptr = ps_mi.tile([128, 8, 128], F16, tag="mi", name="ps_qT")
            for t in range(8):
                nc.tensor.transpose(ptr[:, t, :],
                                    qf[:, t * 128:(t + 1) * 128], ident[:])
            nc.vector.tensor_copy(st[m]["qT"][:, :, sub * 128:(sub + 1) * 128], ptr[:])

        def c_mm(m, sub):
            qT = st[m]["qT"]
            ps_ns = ps_mi.tile([128, HEADS], F32, tag="mi", name="ps_ns")
            for ch in range(8):
                nc.tensor.matmul(
                    ps_ns[:, 2 * ch:2 * ch + 2],
                    lhsT=qT[:, ch, sub * 128:(sub + 1) * 128],
                    rhs=knull_sb[:, :], start=True,
                    stop=True, skip_group_check=True)
            st[m][f"ns{sub}"] = ps_ns

        def c_exp(m, sub):
            en = enp.tile([128, HEADS], F16, tag="en", name="en")
            nc.scalar.activation(out=en[:], in_=st[m][f"ns{sub}"], func=AF.Exp,
                                 bias=expb[:], scale=SCALE)
            st[m][f"en{sub}"] = en

        def c_t(m, sub):
            ps_ent = ps_mi.tile([HEADS, 128], F16, tag="mi", name="ps_ent")
            nc.tensor.transpose(ps_ent[:], st[m][f"en{sub}"], ident[:])
            enT = enTs.tile([HEADS, 128], F16, tag="enT", name="enT")
            nc.vector.tensor_copy(enT[:], ps_ent[:])
            st[m][f"enT{sub}"] = enT

        def d_stage(m, h0, h1):
            if h0 == 0:
                st[m]["et"] = etp.tile([128, 2, HEADS, MACRO], F16, tag="et",
                                       name="et")
            et, qT = st[m]["et"], st[m]["qT"]
            for h in range(h0, h1):
                jb, ch = (h % 2) * HD, h // 2
                sc = ps_pk.tile([128, 2, MACRO], F32, tag="pk", name="sc")
                for kb in range(2):
                    nc.tensor.matmul(
                        sc[:, kb, :],
                        lhsT=kT[jb:jb + HD, ch, kb * 128:(kb + 1) * 128],
                        rhs=qT[jb:jb + HD, ch, :], start=True,
                        stop=True, skip_group_check=True)
                nc.scalar.activation(out=et[:, :, h, :], in_=sc[:], func=AF.Exp,
                                     bias=expb[:], scale=SCALE)

        def e_av(m, sub):
            et, enT = st[m]["et"], st[m][f"enT{sub}"]
            zn = ps_mi.tile([128, HEADS], F32, tag="mi", name="zn")
            for h in range(HEADS):
                for kb in range(2):
                    nc.tensor.matmul(
                        zn[:, h:h + 1],
                        lhsT=et[:, kb, h, sub * 128:(sub + 1) * 128],
                        rhs=ones_col[:, :], start=(kb == 0),
                        stop=(kb == 1), skip_group_check=True)
            z = rzp.tile([128, HEADS], F32, tag="z", name="z")
            nc.vector.tensor_add(z[:], zn[:], st[m][f"en{sub}"])
            rz = rzp.tile([128, HEADS], F32, tag="rz", name="rz")
            nc.vector.reciprocal(rz[:], z[:])
            ao = aop.tile([128, HEADS, HD], F16, tag="ao", name="ao")
            for half in range(2):
                po = ps_po.tile([128, 8, HD], F32, tag="po", name=f"po{half}")
                for hh in range(8):
                    h = half * 8 + hh
                    for kb in range(2):
                        nc.tensor.matmul(
                            po[:, hh, :],
                            lhsT=et[:, kb, h, sub * 128:(sub + 1) * 128],
                            rhs=v_sb[:, kb, h, :], start=(kb == 0),
                            stop=False, skip_group_check=True)
                    nc.tensor.matmul(
                        po[:, hh, :], lhsT=enT[:, :],
                        rhs=vnd_sb[0:HEADS, h * HD:(h + 1) * HD],
                        start=False, stop=True, skip_group_check=True)
                nc.vector.tensor_mul(
                    ao[:, half * 8:(half + 1) * 8, :], po[:],
                    rz[:, half * 8:(half + 1) * 8].unsqueeze(-1)
                    .broadcast_to([128, 8, HD]))
            st[m][f"ao{sub}"] = ao

        def e_t(m, sub):
            ao_flat = st[m][f"ao{sub}"][:].rearrange("p h d -> p (h d)")
            ptr = ps_mi.tile([128, 8, 128], F16, tag="mi", name="ps_aT")
            for t in range(8):
                nc.tensor.transpose(ptr[:, t, :],
                                    ao_flat[:, t * 128:(t + 1) * 128], ident[:])
            aT = aTp.tile([128, 8, 128], F16, tag="aT", name="aT")
            nc.vector.tensor_copy(aT[:], ptr[:])
            st[m][f"aT{sub}"] = aT

        def f_stage(m, sub):
            aT = st[m][f"aT{sub}"]
            r0 = m * MACRO + sub * 128
            pf = [ps_pk.tile([128, 512], F32, tag="pk", name=f"pf{w}")
                  for w in range(2)]
            for half in range(2):
                for kc in range(8):
                    nc.tensor.matmul(
                        pf[half][:], lhsT=aT[:, kc, :],
                        rhs=wo_sb[:, kc, half * 512:(half + 1) * 512],
                        start=(kc == 0), stop=(kc == 7))
            rstd, negmr = _emit_ln_stats(nc, small, [pf[0][:], pf[1][:]], eps_tile)
            ob = obp.tile([128, DIM], F16, tag="ob", name="ob")
            for half in range(2):
                nc.scalar.activation(out=ob[:, half * 512:(half + 1) * 512],
                                     in_=pf[half][:], func=AF.Identity,
                                     bias=negmr[:], scale=rstd[:])
                nc.sync.dma_start(
                    out=OUT[r0:r0 + 128, half * 512:(half + 1) * 512],
                    in_=ob[:, half * 512:(half + 1) * 512])

        for t in range(NMACRO + LAG):
            if 0 < t < NMACRO:
                a_dma(t)
            if t == 1:
                for kc in range(8):
                    nc.sync.dma_start(out=wo_sb[:, kc, :],
                                      in_=WO[kc * 128:(kc + 1) * 128, :])
            do_b = 1 <= t <= NMACRO
            do_b3 = 2 <= t <= NMACRO + 1
            do_cd = 2 <= t <= NMACRO + 1
            do_ef = t >= LAG
            if do_b3:
                b3(t - 2, 0)
            if do_ef:
                c_t(t - 3, 0)
                c_t(t - 3, 1)
            if do_b:
                b1(t - 1, 0)
            if do_b3:
                b3(t - 2, 1)
            if do_b:
                b1(t - 1, 1)
            if do_cd:
                c_mm(t - 2, 0)
                c_mm(t - 2, 1)
                c_exp(t - 2, 0)
                c_exp(t - 2, 1)
            if do_b:
                b2(t - 1, 0)
                b2(t - 1, 1)
                b2f(t - 1, 0)
                b2f(t - 1, 1)
            if do_cd:
                d_stage(t - 2, 0, 16)
            if do_ef:
                e_av(t - 3, 0)
                e_t(t - 3, 0)
                e_av(t - 3, 1)
                e_t(t - 3, 1)
            if t < NMACRO:
                a_stats(t)
                a_act(t)
            if do_ef:
                f_stage(t - 3, 0)
                f_stage(t - 3, 1)
            if t - 3 - 1 in st:
                del st[t - 3 - 1]

    nc.compile()
    return nc


_NC_BUILDS = {}
_NC_CACHE = None  # last-used module (kept for external tooling/harness)


def kernel(**inputs):
    global _NC_CACHE

    x = np.asarray(inputs["x"], np.float32)
    context = np.asarray(inputs["context"], np.float32)
    Wq = np.asarray(inputs["Wq"], np.float32)
    Wkv = np.asarray(inputs["Wkv"], np.float32)
    Wo = np.asarray(inputs["Wo"], np.float32)
    null_kv = np.asarray(inputs["null_kv"], np.float32)
    q_scale = np.asarray(inputs["q_scale"], np.float32)
    k_scale = np.asarray(inputs["k_scale"], np.float32)
    lig = np.asarray(inputs["ln_in_g"], np.float32)
    lib = np.asarray(inputs["ln_in_b"], np.float32)
    lcg = np.asarray(inputs["ln_ctx_g"], np.float32)
    lcb = np.asarray(inputs["ln_ctx_b"], np.float32)
    log_ = np.asarray(inputs["ln_out_g"], np.float32)
    lob = np.asarray(inputs["ln_out_b"], np.float32)

    qb_np = (lib @ Wq).astype(np.float32)
    kvb_np = (lcb @ Wkv).astype(np.float32)
    zb = not (np.any(qb_np) or np.any(kvb_np))
    if zb not in _NC_BUILDS:
        _NC_BUILDS[zb] = build_nc(zero_bias=zb)
    nc = _NC_CACHE = _NC_BUILDS[zb]

    qk = q_scale * k_scale
    kn = null_kv[0]
    kn = kn / max(np.sqrt((kn * kn).sum()), NORM_EPS) * qk
    knull = np.zeros((128, 2), np.float16)
    knull[0:HD, 0] = kn.astype(np.float16)
    knull[HD:128, 1] = kn.astype(np.float16)
    vnd = np.zeros((HEADS, DIM), np.float16)
    for h in range(HEADS):
        vnd[h, h * HD:(h + 1) * HD] = null_kv[1].astype(np.float16)

    shared = {
        "Wq": (lig[:, None] * Wq).astype(np.float16),
        "qb": qb_np[None, :].astype(np.float16),
        "Wkv": (lcg[:, None] * Wkv).astype(np.float16),
        "kvb": kvb_np[None, :].astype(np.float16),
        "Wo": Wo.astype(np.float16),
        "qk": qk,
        "knull": knull,
        "vnd": vnd,
    }
    B, N, _ = x.shape
    in_maps = []
    for c in range(N_CORES):
        b, n0 = c // 2, (c % 2) * ROWS
        in_maps.append({"xs": np.ascontiguousarray(x[b, n0:n0 + ROWS]),
                        "ctx": np.ascontiguousarray(context[b]), **shared})

    res = run_bass_kernel_spmd(nc, in_maps, list(range(N_CORES)))

    out = np.empty((B, N, DIM), np.float32)
    for c in range(N_CORES):
        b, n0 = c // 2, (c % 2) * ROWS
        out[b, n0:n0 + ROWS] = res.results[c]["out"].astype(np.float32)
    # ln_out gain/bias applied host-side (identity for g=1, b=0).
    if not (np.all(log_ == 1.0) and np.all(lob == 0.0)):
        out = out * log_ + lob
    return out
